# revision 3
# baseline (speedup 1.0000x reference)
# DTNN (gnn_message_passing) Trainium2 Bass kernel.
#
# Sharding: data-parallel over batch B=32 across 8 NeuronCores (4 molecules
# per core); the small weight matrices are replicated to every core.
#
# Per-core layout (molecule m, row r = i*64+j):
#   Ct    [101, 4096] fp16  = C[m].T with a trailing ones-row (folds bc into MM1)
#   fC^T  [2][128, 4096] fp16, f on partitions (two halves of NF=256) - resident,
#         computed once, reused by all 3 interaction passes
#   pass p: fX^T = Wi_h.T @ X^T (PE) -> (+bi)*colmask (DVE)
#           fVj^T = fC^T * bcast_i(fX^T)  (DVE, fp16 2x mode)
#           Vj^T  = sum_h Wf_h.T @ fVj_h  (PE, PSUM fp32)
#           Vt    = tanh(Vj^T)            (ACT -> SBUF fp16)
#           S     = sum_j Vt              (GPSIMD fold + DVE reduce)
#           X^T  += S - diag(Vt)          (DVE)
#   head:   o1 = tanh(W1.T @ X^T + b1); y = sum_i mask_i * (W2.T @ o1 + b2)
#
# The pairwise mask cm_i*cm_j*(i!=j) is applied as: cm_j folded into fX^T
# (tanh(0)=0 makes masked columns vanish), diagonal term subtracted exactly,
# and cm_i applied only in the final head (X rows of invalid atoms never
# influence valid outputs because their j-columns are masked each pass).

import numpy as np

B, N, NG, NB, NF, MAXZ = 32, 64, 100, 128, 256, 20
NPASS = 3
NCORES = 8
MPC = B // NCORES          # molecules per core
R = N * N                  # 4096 pair-rows per molecule
G1 = NG + 1                # gaussians + ones row
P = 128

_CACHE = {}


def _build_program():
    from contextlib import ExitStack

    import concourse.bass as bass
    import concourse.bacc as bacc
    import concourse.tile as tile
    from concourse import mybir

    f16 = mybir.dt.float16
    f32 = mybir.dt.float32
    ALU = mybir.AluOpType
    TANH = mybir.ActivationFunctionType.Tanh

    nc = bacc.Bacc(
        "TRN2", target_bir_lowering=False, debug=False, num_devices=NCORES
    )

    dram = {}

    def din(name, shape, dt):
        dram[name] = nc.dram_tensor(name, shape, dt, kind="ExternalInput").ap()

    din("ct", [MPC, G1, R], f16)
    din("x0t", [MPC, P, N], f32)
    din("cmr", [MPC, P, N], f16)
    din("am", [MPC, 1, N], f32)
    din("wct", [G1, NF], f16)
    din("wi", [NB, NF], f16)
    din("bi2", [P, 2], f32)
    din("wf", [NF, NB], f16)
    din("w1", [NB, N], f16)
    din("b1", [N, 1], f32)
    din("w2", [N, 1], f16)
    din("b2", [1, 1], f32)
    y_ap = nc.dram_tensor("y", [1, MPC], f32, kind="ExternalOutput").ap()

    def bcast_mid(ap, rep):
        # [P, n] -> [P, rep, n] broadcast view (step-0 middle dim)
        return bass.AP(ap.tensor, ap.offset, [list(ap.ap[0]), [0, rep], list(ap.ap[1])])

    def stride_view(ap, step, count):
        # [P, X] flat -> [P, count] elements at offsets k*step
        return bass.AP(ap.tensor, ap.offset, [list(ap.ap[0]), [step, count]])

    with tile.TileContext(nc) as tc, ExitStack() as ctx:
        wp = ctx.enter_context(tc.tile_pool(name="wp", bufs=1))
        st = ctx.enter_context(tc.tile_pool(name="st", bufs=1))
        ctp = ctx.enter_context(tc.tile_pool(name="ctp", bufs=2))
        fvp = ctx.enter_context(tc.tile_pool(name="fvp", bufs=2))
        vtp = ctx.enter_context(tc.tile_pool(name="vtp", bufs=2))
        h1p = ctx.enter_context(tc.tile_pool(name="h1p", bufs=2))
        sm = ctx.enter_context(tc.tile_pool(name="sm", bufs=3))
        psb = ctx.enter_context(tc.tile_pool(name="psb", bufs=3, space="PSUM"))
        pss = ctx.enter_context(tc.tile_pool(name="pss", bufs=2, space="PSUM"))

        # ---- weights / per-molecule state ------------------------------
        wct_sb = wp.tile([G1, NF], f16, tag="wct")
        nc.sync.dma_start(wct_sb[:], dram["wct"])
        wi_sb = wp.tile([NB, NF], f16, tag="wi")
        nc.sync.dma_start(wi_sb[:], dram["wi"])
        bi2_sb = wp.tile([P, 2], f32, tag="bi2")
        nc.sync.dma_start(bi2_sb[:], dram["bi2"])
        wf_sb = []
        for h in range(2):
            t = wp.tile([NB, NB], f16, tag=f"wf{h}", name=f"wf{h}")
            nc.sync.dma_start(t[:], dram["wf"][NB * h : NB * (h + 1), :])
            wf_sb.append(t)
        w1_sb = wp.tile([NB, N], f16, tag="w1")
        nc.sync.dma_start(w1_sb[:], dram["w1"])
        b1_sb = wp.tile([N, 1], f32, tag="b1")
        nc.sync.dma_start(b1_sb[:], dram["b1"])
        w2_sb = wp.tile([N, 1], f16, tag="w2")
        nc.sync.dma_start(w2_sb[:], dram["w2"])
        b2_sb = wp.tile([1, 1], f32, tag="b2")
        nc.sync.dma_start(b2_sb[:], dram["b2"])

        xt, cm_sb, am_sb = [], [], []
        for m in range(MPC):
            t = st.tile([P, N], f32, tag=f"xt{m}", name=f"xt{m}")
            nc.sync.dma_start(t[:], dram["x0t"][m, :, :])
            xt.append(t)
            t = st.tile([P, N], f16, tag=f"cm{m}", name=f"cm{m}")
            nc.sync.dma_start(t[:], dram["cmr"][m, :, :])
            cm_sb.append(t)
            t = st.tile([1, N], f32, tag=f"am{m}", name=f"am{m}")
            nc.sync.dma_start(t[:], dram["am"][m, :, :])
            am_sb.append(t)
        fc = [
            [st.tile([P, R], f16, tag=f"fc{m}_{h}", name=f"fc{m}_{h}") for h in range(2)]
            for m in range(MPC)
        ]
        ysb = st.tile([1, MPC], f32, tag="ysb")

        # ---- phase A: fC^T = Wct.T @ Ct, PSUM -> SBUF fp16 -------------
        for m in range(MPC):
            ct_sb = ctp.tile([G1, R], f16, tag="ct")
            nc.sync.dma_start(ct_sb[:], dram["ct"][m, :, :])
            for h in range(2):
                for t4 in range(4):
                    ps = psb.tile([P, 1024], f32, tag="big")
                    for s in range(2):
                        col = 1024 * t4 + 512 * s
                        nc.tensor.matmul(
                            ps[:, 512 * s : 512 * (s + 1)],
                            lhsT=wct_sb[:, NB * h : NB * (h + 1)],
                            rhs=ct_sb[:, col : col + 512],
                            start=True,
                            stop=True,
                        )
                    dst = fc[m][h][:, 1024 * t4 : 1024 * (t4 + 1)]
                    if (h + t4) % 2 == 0:
                        nc.vector.tensor_copy(dst, ps[:])
                    else:
                        nc.scalar.copy(out=dst, in_=ps[:])

        # ---- phase B: 3 interaction passes -----------------------------
        for _p in range(NPASS):
            for m in range(MPC):
                x16 = sm.tile([P, N], f16, tag="x16")
                nc.vector.tensor_copy(x16[:], xt[m][:])
                fxm = []
                for h in range(2):
                    psf = pss.tile([P, N], f32, tag="fx")
                    nc.tensor.matmul(
                        psf[:],
                        lhsT=wi_sb[:, NB * h : NB * (h + 1)],
                        rhs=x16[:],
                        start=True,
                        stop=True,
                    )
                    t = sm.tile([P, N], f16, tag=f"fxm{h}", name=f"fxm{h}")
                    nc.vector.scalar_tensor_tensor(
                        out=t[:],
                        in0=psf[:],
                        scalar=bi2_sb[:, h : h + 1],
                        in1=cm_sb[m][:],
                        op0=ALU.add,
                        op1=ALU.mult,
                    )
                    fxm.append(t)
                fvj = []
                for h in range(2):
                    fv = fvp.tile([P, R], f16, tag=f"fvj{h}", name=f"fvj{h}")
                    nc.vector.tensor_mul(
                        fv[:].rearrange("p (i j) -> p i j", j=N),
                        fc[m][h][:].rearrange("p (i j) -> p i j", j=N),
                        bcast_mid(fxm[h][:], N),
                    )
                    fvj.append(fv)
                vjt = vtp.tile([P, R], f16, tag="vjt")
                for t4 in range(4):
                    psv = psb.tile([P, 1024], f32, tag="big")
                    for s in range(2):
                        col = 1024 * t4 + 512 * s
                        for h in range(2):
                            nc.tensor.matmul(
                                psv[:, 512 * s : 512 * (s + 1)],
                                lhsT=wf_sb[h][:],
                                rhs=fvj[h][:, col : col + 512],
                                start=(h == 0),
                                stop=(h == 1),
                            )
                    nc.scalar.activation(
                        out=vjt[:, 1024 * t4 : 1024 * (t4 + 1)],
                        in_=psv[:],
                        func=TANH,
                    )
                # reduce over j (free dim): GPSIMD folds j in half, DVE sums
                vjt3 = vjt[:].rearrange("p (i j) -> p i j", j=N)
                h1 = h1p.tile([P, N, N // 2], f32, tag="h1")
                nc.gpsimd.tensor_add(
                    h1[:], vjt3[:, :, 0 : N // 2], vjt3[:, :, N // 2 : N]
                )
                s32 = sm.tile([P, N], f32, tag="s32")
                nc.vector.reduce_sum(
                    out=s32[:], in_=h1[:], axis=mybir.AxisListType.X
                )
                dvec = sm.tile([P, N], f16, tag="dvec")
                nc.scalar.copy(out=dvec[:], in_=stride_view(vjt[:], N + 1, N))
                vtmp = sm.tile([P, N], f32, tag="vtmp")
                nc.vector.tensor_sub(vtmp[:], s32[:], dvec[:])
                nc.vector.tensor_add(xt[m][:], xt[m][:], vtmp[:])

        # ---- head ------------------------------------------------------
        for m in range(MPC):
            x16 = sm.tile([P, N], f16, tag="x16")
            nc.vector.tensor_copy(x16[:], xt[m][:])
            pso = pss.tile([N, N], f32, tag="fx")
            nc.tensor.matmul(
                pso[:], lhsT=w1_sb[:], rhs=x16[:], start=True, stop=True
            )
            o1t = sm.tile([N, N], f16, tag="o1t")
            nc.scalar.activation(
                out=o1t[:], in_=pso[:], func=TANH, bias=b1_sb[:], scale=1.0
            )
            psy = pss.tile([1, N], f32, tag="fx")
            nc.tensor.matmul(
                psy[:], lhsT=w2_sb[:], rhs=o1t[:], start=True, stop=True
            )
            yrow = sm.tile([1, N], f32, tag="yrow")
            nc.vector.scalar_tensor_tensor(
                out=yrow[:],
                in0=psy[:],
                scalar=b2_sb[0:1, 0:1],
                in1=am_sb[m][:],
                op0=ALU.add,
                op1=ALU.mult,
            )
            nc.vector.reduce_sum(
                out=ysb[0:1, m : m + 1], in_=yrow[:], axis=mybir.AxisListType.X
            )
        nc.sync.dma_start(y_ap, ysb[:])

    nc.compile()
    return nc


def _get_nc():
    if "nc" not in _CACHE:
        _CACHE["nc"] = _build_program()
    return _CACHE["nc"]


def _prep(inputs):
    Z = np.asarray(inputs["Z"], dtype=np.int32)
    C = np.asarray(inputs["C"], dtype=np.float32)
    W_emb = np.asarray(inputs["W_emb"], dtype=np.float32)
    Wc = np.asarray(inputs["Wc"], dtype=np.float32)
    bc = np.asarray(inputs["bc"], dtype=np.float32)
    Wi = np.asarray(inputs["Wi"], dtype=np.float32)
    bi = np.asarray(inputs["bi"], dtype=np.float32)
    Wf = np.asarray(inputs["Wf"], dtype=np.float32)
    W1 = np.asarray(inputs["W1"], dtype=np.float32)
    b1 = np.asarray(inputs["b1"], dtype=np.float32)
    W2 = np.asarray(inputs["W2"], dtype=np.float32)
    b2 = np.asarray(inputs["b2"], dtype=np.float32)

    ct_full = np.empty((B, G1, R), np.float16)
    ct_full[:, :NG, :] = (
        C.transpose(0, 3, 1, 2).reshape(B, NG, R).astype(np.float16)
    )
    ct_full[:, NG, :] = 1.0
    X0T = np.ascontiguousarray(
        W_emb[Z].transpose(0, 2, 1).astype(np.float32)
    )  # [B, NB, N]
    cm = (Z > 0).astype(np.float32)  # [B, N]
    cmr = np.ascontiguousarray(
        np.broadcast_to(cm[:, None, :], (B, P, N)).astype(np.float16)
    )
    am = np.ascontiguousarray(cm.reshape(B, 1, N).astype(np.float32))

    shared = dict(
        wct=np.ascontiguousarray(
            np.concatenate([Wc, bc[None, :]], axis=0).astype(np.float16)
        ),
        wi=Wi.astype(np.float16),
        bi2=np.ascontiguousarray(bi.reshape(2, P).T.astype(np.float32)),
        wf=Wf.astype(np.float16),
        w1=W1.astype(np.float16),
        b1=b1.reshape(N, 1).astype(np.float32),
        w2=W2.astype(np.float16),
        b2=b2.reshape(1, 1).astype(np.float32),
    )
    in_maps = []
    for k in range(NCORES):
        sl = slice(k * MPC, (k + 1) * MPC)
        in_maps.append(
            dict(
                ct=np.ascontiguousarray(ct_full[sl]),
                x0t=np.ascontiguousarray(X0T[sl]),
                cmr=np.ascontiguousarray(cmr[sl]),
                am=np.ascontiguousarray(am[sl]),
                **shared,
            )
        )
    return in_maps


LAST_RESULTS = None


def kernel(**inputs) -> np.ndarray:
    global LAST_RESULTS
    from concourse import bass_utils

    nc = _get_nc()
    in_maps = _prep(inputs)
    res = bass_utils.run_bass_kernel_spmd(
        nc, in_maps, core_ids=list(range(NCORES))
    )
    LAST_RESULTS = res
    y = np.concatenate(
        [r["y"].reshape(MPC) for r in res.results]
    ).reshape(B, 1).astype(np.float32)
    return y
